# revision 1
# baseline (speedup 1.0000x reference)
"""DilatedRNNStack Trainium2 kernel.

Strategy: data-parallel over batch (B=512 -> 64 rows/core on 8 cores).
Feature-major layout on-chip: every activation tile is [features(part), batch(free)].
Gate matmuls: g.T[128,64] = lhsT.T @ rhs with W stationary, batch on the free dim.

State feature order is permuted host-side to h-first ([orig 96:128, orig 0:96]) so:
  - h slices live at partitions 0:32 -> written straight into ring-buffer tiles
  - o slices live at partitions 32:128 -> written into `whole` tiles whose rows
    0:32 are zero; the next layer consumes whole[0:128] as a K=128 matmul chunk
    against weights whose first 32 rows are zero. No partition-shift copies.
Biases ride as an extra constant-1.0 row on the h-delay ring chunk (K=33).

Time loop is a software-pipelined wavefront: at tick s, layer l works on t=s-l,
so each cross-layer edge has one full tick of slack. Rings are indexed t%d which
is static inside a 12-step-unrolled For_i body (12 = lcm of dilations 1,3,6,12).
"""

import numpy as np

T, B, BSH = 512, 512, 64
DIL = (1, 3, 6, 12)
NCHUNK = 43  # ceil(515/12): x/y staged in 12-step chunks of [64, 12*64]

_prog = None


def _build():
    global _prog
    if _prog is not None:
        return _prog
    import concourse.bass as bass
    import concourse.tile as tile
    from concourse import bacc, mybir

    f32 = mybir.dt.float32
    Tanh = mybir.ActivationFunctionType.Tanh
    Sig = mybir.ActivationFunctionType.Sigmoid

    nc = bacc.Bacc("TRN2", target_bir_lowering=False, debug=False, num_devices=8)
    x_ap = nc.dram_tensor("x", [NCHUNK, 64, 768], f32, kind="ExternalInput").ap()
    y_ap = nc.dram_tensor("y", [NCHUNK, 64, 768], f32, kind="ExternalOutput").ap()
    wA_ap = [nc.dram_tensor(f"wA{l}", [64 if l == 0 else 128, 512], f32,
                            kind="ExternalInput").ap() for l in range(4)]
    wB20_ap = nc.dram_tensor("wB20", [33, 512], f32, kind="ExternalInput").ap()
    wP_ap = [None] + [nc.dram_tensor(f"wP{l}", [65, 512], f32,
                                     kind="ExternalInput").ap() for l in (1, 2, 3)]
    wY_ap = nc.dram_tensor("wY", [128, 64], f32, kind="ExternalInput").ap()

    with tile.TileContext(nc) as tc:
        import contextlib
        ctx = contextlib.ExitStack()
        with ctx:
            wpool = ctx.enter_context(tc.tile_pool(name="w", bufs=1))
            state = ctx.enter_context(tc.tile_pool(name="state", bufs=1))
            xin = ctx.enter_context(tc.tile_pool(name="xin", bufs=3))
            gpool = ctx.enter_context(tc.tile_pool(name="gates", bufs=8))
            tpool = ctx.enter_context(tc.tile_pool(name="tmp", bufs=24))
            spool = ctx.enter_context(tc.tile_pool(name="stage", bufs=2))
            ypool = ctx.enter_context(tc.tile_pool(name="yout", bufs=3))
            pgate = ctx.enter_context(tc.tile_pool(name="psg", bufs=6, space="PSUM"))
            py = ctx.enter_context(tc.tile_pool(name="psy", bufs=1, space="PSUM"))

            # --- load weights ---
            wA = [wpool.tile([64 if l == 0 else 128, 512], f32, name=f"wA{l}", tag=f"wA{l}")
                  for l in range(4)]
            wB20 = wpool.tile([33, 512], f32, name="wB20", tag="wB20")
            wP = [None] + [wpool.tile([65, 512], f32, name=f"wP{l}", tag=f"wP{l}") for l in (1, 2, 3)]
            wY = wpool.tile([128, 64], f32, name="wY", tag="wY")
            nc.sync.dma_start(wB20, wB20_ap)
            for l in range(4):
                nc.sync.dma_start(wA[l], wA_ap[l])
                if l > 0:
                    nc.sync.dma_start(wP[l], wP_ap[l])
            nc.sync.dma_start(wY, wY_ap)

            # --- persistent state ---
            # h rings: rows 0:32 = h, row 32 = 1.0 (bias row). c rings [128, 64*d].
            hr = [state.tile([33, 64 * d], f32, name=f"hr{l}", tag=f"hr{l}") for l, d in enumerate(DIL)]
            cr = [state.tile([128, 64 * d], f32, name=f"cr{l}", tag=f"cr{l}") for l, d in enumerate(DIL)]
            # whole tiles (layers 0..2 feed next layer / shortcut), triple-buffered
            whole = [[state.tile([128, 64], f32, name=f"wh{l}_{p}", tag=f"wh{l}_{p}") for p in range(3)]
                     for l in range(3)]
            for l in range(4):
                nc.vector.memset(hr[l][0:32, :], 0.0)
                nc.vector.memset(hr[l][32:33, :], 1.0)
                nc.vector.memset(cr[l], 0.0)
            for l in range(3):
                for p in range(3):
                    nc.vector.memset(whole[l][p][0:32, :], 0.0)
            # pair tiles for layers 1..3: rows 0:32 = h(t-1), 32:64 = h(t-d), 64 = 1.0
            pp = [None] + [[state.tile([65, 64], f32, name=f"pp{l}_{p}", tag=f"pp{l}_{p}")
                            for p in range(2)] for l in (1, 2, 3)]
            for l in (1, 2, 3):
                for p in range(2):
                    nc.vector.memset(pp[l][p][0:32, :], 0.0)
                    nc.vector.memset(pp[l][p][32:64, :], 0.0)
                    nc.vector.memset(pp[l][p][64:65, :], 1.0)

            def tmp():
                return tpool.tile([128, 64], f32, name="ct", tag="ct")

            def cell(l, t, u, xt, st):
                """Emit layer-l cell for step t; u = tick%12 (x slot / stage slot)."""
                d = DIL[l]
                sc = (t % d) * 64
                sp = ((t - 1) % d) * 64
                sdel = sc if t >= d else sp
                ps = pgate.tile([128, 256], f32, name="ps", tag="ps")
                for gi in range(4):
                    o = ps[:, 64 * gi:64 * gi + 64]
                    gsl = slice(128 * gi, 128 * gi + 128)
                    if l == 0:
                        nc.tensor.matmul(o, wA[0][:, gsl], xt[:, 64 * u:64 * u + 64],
                                         start=True, stop=False)
                        nc.tensor.matmul(o, wB20[:, gsl], hr[0][0:33, 0:64],
                                         start=False, stop=True)
                    else:
                        nc.tensor.matmul(o, wA[l][:, gsl], whole[l - 1][t % 3][:, :],
                                         start=True, stop=False)
                        nc.tensor.matmul(o, wP[l][:, gsl], pp[l][t % 2][:, :],
                                         start=False, stop=True)
                g = gpool.tile([128, 256], f32, name="g", tag="g")
                cslot = cr[l][:, sc:sc + 64]
                if t == 0:
                    nc.scalar.activation(cslot, ps[:, 0:64], Tanh)
                    nc.scalar.activation(g[:, 64:256], ps[:, 64:256], Sig)
                else:
                    nc.scalar.activation(g[:, 0:64], ps[:, 0:64], Tanh)
                    nc.scalar.activation(g[:, 64:256], ps[:, 64:256], Sig)
                    cand = g[:, 0:64]
                    f_ = g[:, 64:128]
                    al = g[:, 128:192]
                    pc = cr[l][:, sp:sp + 64]
                    if t >= d and d > 1:
                        dc = cr[l][:, sc:sc + 64]
                        t1 = tmp()
                        nc.vector.tensor_sub(t1, pc, dc)
                        t2 = tmp()
                        nc.vector.tensor_mul(t2, al, t1)
                        t3 = tmp()
                        nc.vector.tensor_add(t3, t2, dc)  # weighted
                        t4 = tmp()
                        nc.vector.tensor_sub(t4, t3, cand)
                    else:
                        t4 = tmp()
                        nc.vector.tensor_sub(t4, pc, cand)
                    t5 = tmp()
                    nc.vector.tensor_mul(t5, f_, t4)
                    nc.vector.tensor_add(cslot, t5, cand)  # new_c -> ring
                # wh: h part (partitions 0:32) -> h ring slot
                eng_wh = nc.vector if l == 0 else nc.gpsimd
                eng_wh.tensor_mul(hr[l][0:32, sc:sc + 64], g[0:32, 192:256],
                                  cr[l][0:32, sc:sc + 64])
                if l > 0:
                    # assemble pair tile for step t+1: prev = h(t) (plain copy),
                    # del = h(t+1-d) (partition-shift SBUF->SBUF DMA, slack d-1 steps)
                    t1 = t + 1
                    np_ = pp[l][t1 % 2]
                    sdn = ((t1 % d) if t1 >= d else ((t1 - 1) % d)) * 64
                    nc.gpsimd.tensor_copy(np_[0:32, :], hr[l][0:32, sc:sc + 64])
                    nc.sync.dma_start(np_[32:64, :], hr[l][0:32, sdn:sdn + 64])
                # wo: o part (partitions 32:128)
                if l == 3:
                    wo3 = tmp()
                    for lo, hi in ((32, 64), (64, 128)):
                        nc.vector.tensor_mul(wo3[lo:hi, :], g[lo:hi, 192:256],
                                             cr[l][lo:hi, sc:sc + 64])
                        nc.vector.tensor_add(st[lo:hi, 64 * u:64 * u + 64],
                                             wo3[lo:hi, :],
                                             whole[1][t % 3][lo:hi, :])
                else:
                    for lo, hi in ((32, 64), (64, 128)):
                        nc.vector.tensor_mul(whole[l][t % 3][lo:hi, :],
                                             g[lo:hi, 192:256],
                                             cr[l][lo:hi, sc:sc + 64])

            def tick(s, u, xt, st):
                for l in range(4):
                    t = s - l
                    if 0 <= t <= T - 1:
                        cell(l, t, u, xt, st)

            def emit_y(st, ncols):
                psy = py.tile([64, 768], f32, name="psy", tag="psy")
                nc.tensor.matmul(psy[:, 0:512], wY, st[:, 0:512], start=True, stop=True)
                if ncols > 512:
                    nc.tensor.matmul(psy[:, 512:768], wY, st[:, 512:768],
                                     start=True, stop=True)
                yt = ypool.tile([64, 768], f32, name="yt", tag="yt")
                nc.scalar.copy(yt[:, 0:ncols], psy[:, 0:ncols])
                return yt

            def new_stage(full_zero):
                st = spool.tile([128, 768], f32, name="st", tag="st")
                nc.vector.memset(st[0:32, :], 0.0)
                nc.vector.memset(st[0:1, :], 1.0)  # bias row (after zeroing 0:32)
                if full_zero:
                    nc.vector.memset(st[32:64, :], 0.0)
                    nc.vector.memset(st[64:128, :], 0.0)
                return st

            # ---- prologue: ticks 0..23 (chunks 0 and 1, static) ----
            for ch in range(2):
                xt = xin.tile([64, 768], f32, name="xt", tag="xt")
                nc.sync.dma_start(xt, x_ap[ch:ch + 1])
                st = new_stage(full_zero=(ch == 0))
                for u in range(12):
                    tick(12 * ch + u, u, xt, st)
                yt = emit_y(st, 768)
                nc.sync.dma_start(y_ap[ch:ch + 1], yt)

            # ---- steady loop: ticks 24..503 (chunks 2..41) ----
            # Inside the body only tick%12 == u is known; all ring/parity mods
            # use a steady representative t = 48+u-l, which matches the real
            # t = 12*i+u-l mod every d (12 % d == 0) and mod 3 (48 % 3 == 0).
            with tc.For_i(2, 42) as iv:
                xt = xin.tile([64, 768], f32, name="xt", tag="xt")
                nc.sync.dma_start(xt, x_ap[bass.ds(iv, 1)])
                st = new_stage(full_zero=False)
                for u in range(12):
                    for l in range(4):
                        cell(l, 48 + u - l, u, xt, st)
                yt = emit_y(st, 768)
                nc.sync.dma_start(y_ap[bass.ds(iv, 1)], yt)

            # ---- epilogue: ticks 504..514 (chunk 42) ----
            xt = xin.tile([64, 768], f32, name="xt", tag="xt")
            nc.sync.dma_start(xt, x_ap[42:43])
            st = new_stage(full_zero=True)
            for u in range(11):
                tick(504 + u, u, xt, st)
            yt = emit_y(st, 768)
            nc.sync.dma_start(y_ap[42:43], yt)

    nc.compile()
    _prog = nc
    return nc


def _prep_weights(ws, bs, Wa, ba):
    PERM = np.r_[96:128, 0:96]
    GORD = [1, 0, 2, 3]  # psum order: cand, forget(+1), alpha, outgate
    ins = [64, 96, 96, 96]
    out = {}
    for l in range(4):
        W, b = ws[l], bs[l]
        Wg = W.reshape(4, 128, -1)[GORD][:, PERM, :]  # [4,128,fan]
        bg = b.reshape(4, 128)[GORD][:, PERM].copy()
        bg[1] += 1.0
        n = ins[l]
        if l == 0:
            A = np.zeros((64, 512), np.float32)
            B2 = np.zeros((33, 512), np.float32)
            for gi in range(4):
                A[:, 128 * gi:128 * gi + 128] = Wg[gi, :, 0:64].T
                B2[0:32, 128 * gi:128 * gi + 128] = (
                    Wg[gi, :, 64:96] + Wg[gi, :, 96:128]).T
                B2[32, 128 * gi:128 * gi + 128] = bg[gi]
            out["wA0"], out["wB20"] = A, B2
        else:
            A = np.zeros((128, 512), np.float32)
            P = np.zeros((65, 512), np.float32)
            for gi in range(4):
                A[32:128, 128 * gi:128 * gi + 128] = Wg[gi, :, 0:96].T
                P[0:32, 128 * gi:128 * gi + 128] = Wg[gi, :, 96:128].T
                P[32:64, 128 * gi:128 * gi + 128] = Wg[gi, :, 128:160].T
                P[64, 128 * gi:128 * gi + 128] = bg[gi]
            out[f"wA{l}"], out[f"wP{l}"] = A, P
    WY = np.zeros((128, 64), np.float32)
    WY[0] = ba
    WY[32:128] = Wa.T
    out["wY"] = WY
    return out


def _run(inputs, trace=False):
    from concourse.bass_utils import run_bass_kernel_spmd

    x = np.ascontiguousarray(np.asarray(inputs["x"], dtype=np.float32))
    ws = [np.asarray(inputs[f"W{l}"], np.float32) for l in range(4)]
    bs = [np.asarray(inputs[f"b{l}"], np.float32) for l in range(4)]
    Wa = np.asarray(inputs["Wa"], np.float32)
    ba = np.asarray(inputs["ba"], np.float32)

    wmap = _prep_weights(ws, bs, Wa, ba)
    nc = _build()

    in_maps = []
    for c in range(8):
        xc = x[:, BSH * c:BSH * c + BSH, :].transpose(0, 2, 1)  # [512, 64f, 64b]
        xp = np.concatenate([xc, np.zeros((NCHUNK * 12 - T, 64, 64), np.float32)])
        xdev = np.ascontiguousarray(
            xp.reshape(NCHUNK, 12, 64, 64).transpose(0, 2, 1, 3).reshape(NCHUNK, 64, 768))
        in_maps.append({"x": xdev, **wmap})

    res = run_bass_kernel_spmd(nc, in_maps, list(range(8)), trace=trace)

    y = np.empty((T, B, 64), np.float32)
    for c in range(8):
        ydev = res.results[c]["y"]  # [43, 64, 768]
        z = ydev.reshape(NCHUNK, 64, 12, 64).transpose(0, 2, 3, 1).reshape(NCHUNK * 12, 64, 64)
        y[:, BSH * c:BSH * c + BSH, :] = z[3:3 + T]  # skew: y(t) at tick t+3
    return y, res


def _time_exec(nc, in_maps, iters=20):
    """Steady-state wall-clock of the compiled NEFF via a reusable jitted fn."""
    import time
    import jax
    import jax.numpy as jnp
    from jax.sharding import Mesh, PartitionSpec
    from jax.experimental.shard_map import shard_map
    from concourse import bass2jax, mybir

    bass2jax.install_neuronx_cc_hook()
    n_cores = len(in_maps)
    partition_name = nc.partition_id_tensor.name if nc.partition_id_tensor else None
    in_names, out_names, out_avals, zero_outs = [], [], [], []
    for alloc in nc.m.functions[0].allocations:
        if not isinstance(alloc, mybir.MemoryLocationSet):
            continue
        name = alloc.memorylocations[0].name
        if alloc.kind == "ExternalInput":
            if name != partition_name:
                in_names.append(name)
        elif alloc.kind == "ExternalOutput":
            shape = list(alloc.tensor_shape)
            npdt = mybir.dt.np(alloc.dtype)
            out_avals.append(jax.core.ShapedArray(shape, npdt))
            out_names.append(name)
            zero_outs.append(np.zeros(shape, npdt))

    n_params = len(in_names)
    n_outs = len(out_names)
    all_in_names = in_names + out_names
    if partition_name is not None:
        all_in_names = all_in_names + [partition_name]
    donate = tuple(range(n_params, n_params + n_outs))

    def _body(*args):
        operands = list(args)
        if partition_name is not None:
            operands.append(bass2jax.partition_id_tensor())
        return tuple(bass2jax._bass_exec_p.bind(
            *operands, out_avals=tuple(out_avals), in_names=tuple(all_in_names),
            out_names=tuple(out_names), lowering_input_output_aliases=(),
            sim_require_finite=True, sim_require_nnan=True, nc=nc))

    devices = jax.devices()[:n_cores]
    mesh = Mesh(np.asarray(devices), ("core",))
    nin = n_params + n_outs
    sharded = jax.jit(shard_map(
        _body, mesh=mesh, in_specs=(PartitionSpec("core"),) * nin,
        out_specs=(PartitionSpec("core"),) * n_outs, check_rep=False),
        donate_argnums=donate, keep_unused=True)
    concat_in = [np.concatenate([m[name] for m in in_maps], axis=0)
                 for name in in_names]
    concat_zeros = [np.zeros((n_cores * z.shape[0], *z.shape[1:]), z.dtype)
                    for z in zero_outs]
    in_args = [jax.device_put(a) for a in concat_in]
    zouts = [jax.device_put(a) for a in concat_zeros]
    out = sharded(*in_args, *zouts)
    jax.block_until_ready(out)
    times = []
    for _ in range(iters):
        # recycle outputs as the donated out-buffers (kernel writes all of y)
        t0 = time.perf_counter()
        out = sharded(*in_args, *list(out))
        jax.block_until_ready(out)
        times.append(time.perf_counter() - t0)
    return min(times), times


def kernel(**inputs):
    y, _ = _run(inputs, trace=False)
    return y



# revision 37
# speedup vs baseline: 18.4768x; 18.4768x over previous
"""DilatedRNNStack Trainium2 kernel (bf16 redesign).

Strategy: data-parallel over batch (B=512 -> 64 rows/core on 8 cores).
Feature-major on-chip: activations are [features(part), batch(free)], all bf16.

Key transforms vs a direct port:
  - alpha-gate weights negated so sigmoid yields (1-alpha) directly:
      c(t) = cand + f * (w - cand),  w = pc + (1-alpha)*(dc - pc)
    One tanh (cand) + one sigmoid (al,f,o) act op per cell-group per tick.
  - whole is stored halved via the fused (c*0.5)*o scalar_tensor_tensor and
    all consumers' weights are doubled host-side (free halving).
  - h is rows 0:32 of whole' -> no separate h ring; prev/del-h matmul terms
    read the whole-history tiles directly (zero-padded weight rows kill the
    o-part where needed).
  - biases enter PSUM as K=1 outer-product matmuls (bias_row x ones);
    each gate-block's terms form one CONTIGUOUS start..stop PSUM group
    (interleaved groups lose writes; stop clears has_written).

Cells are processed in two groups {l0,l1}, {l2,l3} per tick (wavefront skew 1:
layer l works on t=s-l at tick s). Each group: 12 matmuls + (K=1) bias mms,
one sigmoid [128,512], 5 tensor_tensor + 1 scalar_tensor_tensor on DVE, and
2 gpsimd subs. History rings C/W are 12 deep so all slot indices are static
inside a 12-tick For_i body.
"""

import numpy as np

T, B, BSH = 512, 512, 64
DIL = (1, 3, 6, 12)
NCHUNK = 43  # ceil(515/12): x/y staged in 12-step chunks of [64, 12*64]

_prog = {}


def _build(static_steady=None, repeat=None, debug_dump=False):
    """static_steady=N: emit chunks 2..2+N-1 as a static python loop instead
    of the For_i hardware loop (for offline TimelineSim profiling only).
    repeat=R: wrap the whole kernel body in a hardware loop that executes it
    R times back-to-back (for steady-state device timing: the measured slope
    over R removes host/dispatch overhead from the estimate)."""
    global _prog
    key = (static_steady, repeat, debug_dump)
    if key in _prog:
        return _prog[key]
    import concourse.bass as bass
    import concourse.tile as tile
    from concourse import bacc, mybir

    f32 = mybir.dt.float32
    bf16 = mybir.dt.bfloat16
    Sig = mybir.ActivationFunctionType.Sigmoid
    Alu = mybir.AluOpType

    nc = bacc.Bacc("TRN2", target_bir_lowering=False, debug=False, num_devices=8)
    x_ap = nc.dram_tensor("x", [NCHUNK, 64, 768], bf16, kind="ExternalInput").ap()
    y_ap = nc.dram_tensor("y", [NCHUNK, 64, 768], f32, kind="ExternalOutput").ap()
    wX0_ap = nc.dram_tensor("wX0", [64, 512], bf16, kind="ExternalInput").ap()
    wB0_ap = nc.dram_tensor("wB0", [32, 512], bf16, kind="ExternalInput").ap()
    wIn_ap = [None] + [nc.dram_tensor(f"wIn{l}", [128, 512], bf16,
                                      kind="ExternalInput").ap() for l in (1, 2, 3)]
    wPv_ap = [None] + [nc.dram_tensor(f"wPv{l}", [32, 512], bf16,
                                      kind="ExternalInput").ap() for l in (1, 2, 3)]
    wDl_ap = [None] + [nc.dram_tensor(f"wDl{l}", [32, 512], bf16,
                                      kind="ExternalInput").ap() for l in (1, 2, 3)]
    bT_ap = [nc.dram_tensor(f"bT{l}", [1, 512], bf16, kind="ExternalInput").ap()
             for l in range(4)]
    wY_ap = nc.dram_tensor("wY", [128, 64], bf16, kind="ExternalInput").ap()

    with tile.TileContext(nc) as tc:
        import contextlib
        ctx = contextlib.ExitStack()
        with ctx:
            wpool = ctx.enter_context(tc.tile_pool(name="w", bufs=1))
            state = ctx.enter_context(tc.tile_pool(name="state", bufs=1))
            xin = ctx.enter_context(tc.tile_pool(name="xin", bufs=3))
            gpool = ctx.enter_context(tc.tile_pool(name="gates", bufs=4))
            tpool = ctx.enter_context(tc.tile_pool(name="tmp", bufs=8))
            ypool = ctx.enter_context(tc.tile_pool(name="yout", bufs=3))
            pgate = ctx.enter_context(tc.tile_pool(name="psg", bufs=2, space="PSUM"))
            py = ctx.enter_context(tc.tile_pool(name="psy", bufs=1, space="PSUM"))

            if repeat is not None:
                ctx.enter_context(tc.For_i(0, repeat, name="rep"))

            # --- load weights ---
            wX0 = wpool.tile([64, 512], bf16, name="wX0", tag="wX0")
            wB0 = wpool.tile([32, 512], bf16, name="wB0", tag="wB0")
            wIn = [None] + [wpool.tile([128, 512], bf16, name=f"wIn{l}", tag=f"wIn{l}")
                            for l in (1, 2, 3)]
            wPv = [None] + [wpool.tile([32, 512], bf16, name=f"wPv{l}", tag=f"wPv{l}")
                            for l in (1, 2, 3)]
            wDl = [None] + [wpool.tile([32, 512], bf16, name=f"wDl{l}", tag=f"wDl{l}")
                            for l in (1, 2, 3)]
            bT = [wpool.tile([1, 512], bf16, name=f"bT{l}", tag=f"bT{l}")
                  for l in range(4)]
            wY = wpool.tile([128, 64], bf16, name="wY", tag="wY")
            nc.sync.dma_start(wX0, wX0_ap)
            nc.sync.dma_start(wB0, wB0_ap)
            for l in (1, 2, 3):
                nc.sync.dma_start(wIn[l], wIn_ap[l])
                nc.sync.dma_start(wPv[l], wPv_ap[l])
                nc.sync.dma_start(wDl[l], wDl_ap[l])
            for l in range(4):
                nc.sync.dma_start(bT[l], bT_ap[l])
            nc.sync.dma_start(wY, wY_ap)

            # --- persistent state ---
            ones1 = state.tile([1, 64], bf16, name="ones1", tag="ones1")
            nc.vector.memset(ones1, 1.0)
            # history rings, 12 deep, per group of 2 cells: [128, 2*64]
            C = [[state.tile([128, 128], bf16, name=f"C{g}_{k}", tag=f"C{g}_{k}")
                  for k in range(12)] for g in range(2)]
            W = [[state.tile([128, 128], bf16, name=f"W{g}_{k}", tag=f"W{g}_{k}")
                  for k in range(12)] for g in range(2)]
            for g in range(2):
                for k in range(12):
                    nc.vector.memset(C[g][k], 0.0)   # c = 0
                    nc.vector.memset(W[g][k], 0.0)   # whole' (h=0)
            # y stage: row0 = 1.0 (bias), rows 1:32 = 0, rows 32:128 written
            st = state.tile([128, 768], bf16, name="st", tag="st")
            nc.vector.memset(st[0:32, :], 0.0)
            nc.vector.memset(st[0:1, :], 1.0)

            GCELLS = ((0, 1), (2, 3))
            # gate block order in PSUM/g: cand (tanh), al, f, o (sigmoid)
            CD, AL, FG, OG = 0, 1, 2, 3
            Tanh = mybir.ActivationFunctionType.Tanh

            def mm_block(l, t, s, u, xt, ps, ci, gi):
                """One gate-block's full accumulation, CONTIGUOUS start..stop:
                a second start=True abandons an open group's pending writes,
                and stop=True clears has_written (no accumulating onto a
                closed group) — so all of a block's terms must stay adjacent."""
                o = ps[:, 128 * gi + 64 * ci:128 * gi + 64 * ci + 64]
                gsl = slice(128 * gi, 128 * gi + 128)
                k1 = (s - 1) % 12
                grp = l // 2
                gin = (l - 1) // 2   # group holding layer l-1
                cin = (l - 1) % 2
                d = DIL[l]
                kd = (s - d) % 12 if t >= d else k1
                nc.tensor.matmul(o, bT[l][0:1, gsl], ones1, start=True, stop=False)
                if l == 0:
                    nc.tensor.matmul(o, wX0[:, gsl], xt[:, 64 * u:64 * u + 64],
                                     start=False, stop=False)
                    nc.tensor.matmul(o, wB0[:, gsl], W[0][k1][0:32, 0:64],
                                     start=False, stop=True)
                else:
                    nc.tensor.matmul(o, wDl[l][:, gsl],
                                     W[grp][kd][0:32, 64 * ci:64 * ci + 64],
                                     start=False, stop=False)
                    nc.tensor.matmul(o, wIn[l][:, gsl],
                                     W[gin][k1][:, 64 * cin:64 * cin + 64],
                                     start=False, stop=False)
                    nc.tensor.matmul(o, wPv[l][:, gsl],
                                     W[grp][k1][0:32, 64 * ci:64 * ci + 64],
                                     start=False, stop=True)

            def chain_group(grp, s, ts, g):
                """Steady-state batched chain for a full group (both cells
                active, both t >= 1). ts = (t of cell0, t of cell1)."""
                k, k1 = s % 12, (s - 1) % 12
                z1 = tpool.tile([128, 128], bf16, name="z1", tag="z1")
                for ci in (0, 1):
                    l = GCELLS[grp][ci]
                    t = ts[ci]
                    d = DIL[l]
                    kd = (s - d) % 12 if t >= d else k1
                    sl = slice(64 * ci, 64 * ci + 64)
                    nc.gpsimd.tensor_sub(z1[:, sl], C[grp][kd][:, sl], C[grp][k1][:, sl])
                t2 = tpool.tile([128, 128], bf16, name="t2", tag="t2")
                nc.vector.tensor_mul(t2, g[:, 128:256], z1)        # (1-al)*(dc-pc)
                w2 = tpool.tile([128, 128], bf16, name="w2", tag="w2")
                nc.vector.tensor_add(w2, t2, C[grp][k1])           # + pc
                t4 = tpool.tile([128, 128], bf16, name="t4", tag="t4")
                nc.vector.tensor_sub(t4, w2, g[:, 0:128])          # - cand
                t5 = tpool.tile([128, 128], bf16, name="t5", tag="t5")
                nc.vector.tensor_mul(t5, g[:, 256:384], t4)        # * f
                nc.vector.tensor_add(C[grp][k], t5, g[:, 0:128])   # + cand -> c
                nc.vector.scalar_tensor_tensor(                    # (c*0.5)*o
                    W[grp][k], C[grp][k], 0.5, g[:, 384:512],
                    Alu.mult, Alu.mult)

            def chain_cell(grp, ci, s, t, g):
                """Per-cell chain for prologue/epilogue (partial groups or
                t==0/t<d specials). Slices of the group tiles."""
                l = GCELLS[grp][ci]
                d = DIL[l]
                k, k1 = s % 12, (s - 1) % 12
                sl = slice(64 * ci, 64 * ci + 64)
                gsl = lambda gi: g[:, 128 * gi + 64 * ci:128 * gi + 64 * ci + 64]
                if t == 0:
                    nc.scalar.copy(C[grp][k][:, sl], gsl(CD))      # c2 = s_c
                else:
                    kd = (s - d) % 12 if t >= d else k1
                    z1 = tpool.tile([128, 64], bf16, name="z1c", tag="z1c")
                    nc.gpsimd.tensor_sub(z1, C[grp][kd][:, sl], C[grp][k1][:, sl])
                    t2 = tpool.tile([128, 64], bf16, name="t2c", tag="t2c")
                    nc.vector.tensor_mul(t2, gsl(AL), z1)
                    w2 = tpool.tile([128, 64], bf16, name="w2c", tag="w2c")
                    nc.vector.tensor_add(w2, t2, C[grp][k1][:, sl])
                    t4 = tpool.tile([128, 64], bf16, name="t4c", tag="t4c")
                    nc.vector.tensor_sub(t4, w2, gsl(CD))
                    t5 = tpool.tile([128, 64], bf16, name="t5c", tag="t5c")
                    nc.vector.tensor_mul(t5, gsl(FG), t4)
                    nc.vector.tensor_add(C[grp][k][:, sl], t5, gsl(CD))
                nc.vector.scalar_tensor_tensor(
                    W[grp][k][:, sl], C[grp][k][:, sl], 0.5, gsl(OG),
                    Alu.mult, Alu.mult)

            def emit_out(s, u):
                """st[:, u*64] = whole'(l3, t=s-3) + whole'(l1, same t).
                Partition windows: a span starting at 32 may cover only 32
                partitions, so split 32:64 / 64:128."""
                k, k2 = s % 12, (s - 2) % 12
                for lo, hi in ((32, 64), (64, 128)):
                    nc.vector.tensor_add(st[lo:hi, 64 * u:64 * u + 64],
                                         W[1][k][lo:hi, 64:128],
                                         W[0][k2][lo:hi, 64:128])

            def tick(s, u, xt, steady, dump_g=None):
                acts = []
                pss = []
                gts = []
                for grp in (0, 1):
                    cells = [(ci, GCELLS[grp][ci], s - GCELLS[grp][ci])
                             for ci in (0, 1)]
                    act = [(ci, l, t) for ci, l, t in cells if 0 <= t <= T - 1]
                    acts.append(act)
                    ps = (pgate.tile([128, 512], f32, name=f"ps{grp}", tag=f"ps{grp}")
                          if act else None)
                    pss.append(ps)
                    gts.append(gpool.tile([128, 512], bf16, name=f"g{grp}",
                                          tag=f"g{grp}") if act else None)
                # sigmoid blocks (al,f,o) first per group, then the tanh
                # (cand) block; sigmoid act issues first so the chain's
                # first ops (needing al) start earliest.
                for grp in (0, 1):
                    for gates in ((AL, FG, OG), (CD,)):
                        for gi in gates:
                            for ci, l, t in acts[grp]:
                                mm_block(l, t, s, u, xt, pss[grp], ci, gi)
                    if steady:
                        g = gts[grp]
                        nc.scalar.activation(g[:, 128:512], pss[grp][:, 128:512], Sig)
                        nc.scalar.activation(g[:, 0:128], pss[grp][:, 0:128], Tanh)
                for grp in (0, 1):
                    g = gts[grp]
                    if steady:
                        chain_group(grp, s, (s - GCELLS[grp][0], s - GCELLS[grp][1]), g)
                    else:
                        for ci, l, t in acts[grp]:
                            for gi in range(4):
                                io = slice(128 * gi + 64 * ci, 128 * gi + 64 * ci + 64)
                                nc.scalar.activation(g[:, io], pss[grp][:, io],
                                                     Tanh if gi == CD else Sig)
                        for ci, l, t in acts[grp]:
                            chain_cell(grp, ci, s, t, g)
                    if dump_g is not None and grp == 0 and g is not None:
                        gf = tpool.tile([128, 512], f32, name="gdump", tag="gdump")
                        nc.scalar.copy(gf, g)
                        nc.sync.dma_start(dump_g, gf)
                if 0 <= s - 3 <= T - 1:
                    emit_out(s, u)

            def emit_y(ch):
                psy = py.tile([64, 768], f32, name="psy", tag="psy")
                nc.tensor.matmul(psy[:, 0:512], wY, st[:, 0:512], start=True, stop=True)
                nc.tensor.matmul(psy[:, 512:768], wY, st[:, 512:768],
                                 start=True, stop=True)
                yt = ypool.tile([64, 768], f32, name="yt", tag="yt")
                nc.scalar.copy(yt, psy)
                if isinstance(ch, int):
                    nc.sync.dma_start(y_ap[ch:ch + 1], yt)
                else:
                    nc.sync.dma_start(y_ap[bass.ds(ch, 1)], yt)

            if debug_dump:
                dW_ap = nc.dram_tensor("dW", [2, 12, 128, 128], f32,
                                       kind="ExternalOutput").ap()
                dC_ap = nc.dram_tensor("dC", [2, 12, 128, 128], f32,
                                       kind="ExternalOutput").ap()
                dG_ap = nc.dram_tensor("dG", [128, 512], f32,
                                       kind="ExternalOutput").ap()

            # ---- prologue: ticks 0..23 (chunks 0 and 1, static) ----
            for ch in range(2):
                xt = xin.tile([64, 768], bf16, name="xt", tag="xt")
                nc.sync.dma_start(xt, x_ap[ch:ch + 1])
                for u in range(12):
                    s = 12 * ch + u
                    tick(s, u, xt, steady=(s >= 13),
                         dump_g=(dG_ap if (debug_dump and s == 0) else None))
                emit_y(ch)
                if debug_dump and ch == 0:
                    for g_ in range(2):
                        for k_ in range(12):
                            tf = tpool.tile([128, 128], f32, name="df", tag="df")
                            nc.scalar.copy(tf, W[g_][k_])
                            nc.sync.dma_start(dW_ap[g_:g_ + 1, k_:k_ + 1], tf)
                            tf2 = tpool.tile([128, 128], f32, name="df2", tag="df2")
                            nc.scalar.copy(tf2, C[g_][k_])
                            nc.sync.dma_start(dC_ap[g_:g_ + 1, k_:k_ + 1], tf2)

            # ---- steady loop: ticks 24..503 (chunks 2..41) ----
            # All ring indices use s % 12 == u (12 | chunk stride).
            if static_steady is not None:
                for ci_ in range(2, 2 + static_steady):
                    xt = xin.tile([64, 768], bf16, name="xt", tag="xt")
                    nc.sync.dma_start(xt, x_ap[ci_:ci_ + 1])
                    for u in range(12):
                        tick(12 * ci_ + u, u, xt, steady=True)
                    emit_y(ci_)
            else:
                with tc.For_i(2, 42) as iv:
                    xt = xin.tile([64, 768], bf16, name="xt", tag="xt")
                    nc.sync.dma_start(xt, x_ap[bass.ds(iv, 1)])
                    for u in range(12):
                        tick(48 + u, u, xt, steady=True)
                    emit_y(iv)

            # ---- epilogue: ticks 504..514 (chunk 42) ----
            xt = xin.tile([64, 768], bf16, name="xt", tag="xt")
            nc.sync.dma_start(xt, x_ap[42:43])
            for u in range(11):
                s = 504 + u
                tick(s, u, xt, steady=(s <= 511))
            emit_y(42)

    nc.compile()
    _prog[key] = nc
    return nc


def _prep_weights(ws, bs, Wa, ba):
    import ml_dtypes
    BF = ml_dtypes.bfloat16
    PERM = np.r_[96:128, 0:96]
    GORD = [1, 2, 0, 3]  # block order: cand (tanh), alpha, forget, outgate
    out = {}
    for l in range(4):
        W, b = np.asarray(ws[l], np.float32), np.asarray(bs[l], np.float32)
        Wg = W.reshape(4, 128, -1)[GORD][:, PERM, :].copy()  # [4,128,fan]
        bg = b.reshape(4, 128)[GORD][:, PERM].copy()
        Wg[1] *= -1.0         # alpha negated -> sigmoid gives 1-alpha
        bg[1] *= -1.0
        bg[2] += 1.0          # forget-gate +1
        if l == 0:
            A = np.zeros((64, 512), np.float32)
            Bc = np.zeros((32, 512), np.float32)
            for gi in range(4):
                A[:, 128 * gi:128 * gi + 128] = Wg[gi, :, 0:64].T
                Bc[:, 128 * gi:128 * gi + 128] = (
                    Wg[gi, :, 64:96] + Wg[gi, :, 96:128]).T * 2.0
            out["wX0"], out["wB0"] = A.astype(BF), Bc.astype(BF)
        else:
            A = np.zeros((128, 512), np.float32)
            P = np.zeros((32, 512), np.float32)
            D = np.zeros((32, 512), np.float32)
            for gi in range(4):
                A[32:128, 128 * gi:128 * gi + 128] = Wg[gi, :, 0:96].T * 2.0
                P[:, 128 * gi:128 * gi + 128] = Wg[gi, :, 96:128].T * 2.0
                D[:, 128 * gi:128 * gi + 128] = Wg[gi, :, 128:160].T * 2.0
            out[f"wIn{l}"] = A.astype(BF)
            out[f"wPv{l}"] = P.astype(BF)
            out[f"wDl{l}"] = D.astype(BF)
        out[f"bT{l}"] = np.ascontiguousarray(bg.reshape(1, 512)).astype(BF)
    WY = np.zeros((128, 64), np.float32)
    WY[0] = np.asarray(ba, np.float32)
    WY[32:128] = np.asarray(Wa, np.float32).T * 2.0
    out["wY"] = WY.astype(BF)
    return out


def _run(inputs, trace=False):
    import ml_dtypes
    from concourse.bass_utils import run_bass_kernel_spmd

    BF = ml_dtypes.bfloat16
    x = np.ascontiguousarray(np.asarray(inputs["x"], dtype=np.float32))
    ws = [np.asarray(inputs[f"W{l}"], np.float32) for l in range(4)]
    bs = [np.asarray(inputs[f"b{l}"], np.float32) for l in range(4)]
    Wa = np.asarray(inputs["Wa"], np.float32)
    ba = np.asarray(inputs["ba"], np.float32)

    wmap = _prep_weights(ws, bs, Wa, ba)
    nc = _build()

    in_maps = []
    for c in range(8):
        xc = x[:, BSH * c:BSH * c + BSH, :].transpose(0, 2, 1)  # [512, 64f, 64b]
        xp = np.concatenate([xc, np.zeros((NCHUNK * 12 - T, 64, 64), np.float32)])
        xdev = np.ascontiguousarray(
            xp.reshape(NCHUNK, 12, 64, 64).transpose(0, 2, 1, 3)
            .reshape(NCHUNK, 64, 768)).astype(BF)
        in_maps.append({"x": xdev, **wmap})

    res = run_bass_kernel_spmd(nc, in_maps, list(range(8)), trace=trace)

    y = np.empty((T, B, 64), np.float32)
    for c in range(8):
        ydev = res.results[c]["y"]  # [43, 64, 768]
        z = ydev.reshape(NCHUNK, 64, 12, 64).transpose(0, 2, 3, 1).reshape(NCHUNK * 12, 64, 64)
        y[:, BSH * c:BSH * c + BSH, :] = z[3:3 + T]  # skew: y(t) at tick t+3
    return y, res


def _time_exec(nc, in_maps, iters=20):
    """Steady-state wall-clock of the compiled NEFF via a reusable jitted fn."""
    import time
    import jax
    import jax.numpy as jnp
    from jax.sharding import Mesh, PartitionSpec
    from jax.experimental.shard_map import shard_map
    from concourse import bass2jax, mybir

    bass2jax.install_neuronx_cc_hook()
    n_cores = len(in_maps)
    partition_name = nc.partition_id_tensor.name if nc.partition_id_tensor else None
    in_names, out_names, out_avals, zero_outs = [], [], [], []
    for alloc in nc.m.functions[0].allocations:
        if not isinstance(alloc, mybir.MemoryLocationSet):
            continue
        name = alloc.memorylocations[0].name
        if alloc.kind == "ExternalInput":
            if name != partition_name:
                in_names.append(name)
        elif alloc.kind == "ExternalOutput":
            shape = list(alloc.tensor_shape)
            npdt = mybir.dt.np(alloc.dtype)
            out_avals.append(jax.core.ShapedArray(shape, npdt))
            out_names.append(name)
            zero_outs.append(np.zeros(shape, npdt))

    n_params = len(in_names)
    n_outs = len(out_names)
    all_in_names = in_names + out_names
    if partition_name is not None:
        all_in_names = all_in_names + [partition_name]
    donate = tuple(range(n_params, n_params + n_outs))

    def _body(*args):
        operands = list(args)
        if partition_name is not None:
            operands.append(bass2jax.partition_id_tensor())
        return tuple(bass2jax._bass_exec_p.bind(
            *operands, out_avals=tuple(out_avals), in_names=tuple(all_in_names),
            out_names=tuple(out_names), lowering_input_output_aliases=(),
            sim_require_finite=True, sim_require_nnan=True, nc=nc))

    devices = jax.devices()[:n_cores]
    mesh = Mesh(np.asarray(devices), ("core",))
    nin = n_params + n_outs
    sharded = jax.jit(shard_map(
        _body, mesh=mesh, in_specs=(PartitionSpec("core"),) * nin,
        out_specs=(PartitionSpec("core"),) * n_outs, check_rep=False),
        donate_argnums=donate, keep_unused=True)
    concat_in = [np.concatenate([m[name] for m in in_maps], axis=0)
                 for name in in_names]
    concat_zeros = [np.zeros((n_cores * z.shape[0], *z.shape[1:]), z.dtype)
                    for z in zero_outs]
    in_args = [jax.device_put(a) for a in concat_in]
    zouts = [jax.device_put(a) for a in concat_zeros]
    out = sharded(*in_args, *zouts)
    jax.block_until_ready(out)
    times = []
    for _ in range(iters):
        # recycle outputs as the donated out-buffers (kernel writes all of y)
        t0 = time.perf_counter()
        out = sharded(*in_args, *list(out))
        jax.block_until_ready(out)
        times.append(time.perf_counter() - t0)
    return min(times), times


def kernel(**inputs):
    y, _ = _run(inputs, trace=False)
    return y
